# revision 26
# baseline (speedup 1.0000x reference)
"""Trainium2 Bass kernel for nn_AttentionBlock (B=16, C=256, H=W=32, NH=4, GROUPS=8).

Strategy: data-parallel over batch. 8 cores x 2 batch elements each; no
collectives. Per batch element, [channels, spatial] layout (channels on SBUF
partitions):

  1. GroupNorm: per-channel stats via DVE bn_stats/bn_aggr, group aggregation
     via a tiny matmul against a block-diagonal averaging matrix, rstd via the
     magic-constant rsqrt seed + one Newton iteration (keeps ACT exclusively
     on the exp table). hn written as fp8 [128, ct-plane, s].
  2. qkv 1x1 conv in fp8 DoubleRow (contraction = 2 x 128 channel planes).
     Q and K are produced in a d-split layout: head h occupies partitions
     32h..32h+32 with d = 32z + p across two free-dim planes, so the score
     matmuls can run as fp8 DoubleRow with 32-partition stationaries.
     V is produced transposed into vt [128(k), KT, 4(head), 128] where each
     head block is [V_h(64 cols) | ones(64 cols)] - the ones columns make the
     PV matmul emit the softmax denominator into PSUM partitions 64..127.
  3. Scores: per (t, head-pair, qc) one [128, 1024] two-bank PSUM unit; the
     two heads' DR matmuls ([32, 2, 128] x [32, 2, 512]) sit at PE array row
     positions 64hp and 64hp+32 and stream concurrently. exp on ACT reads the
     full wide unit (scale=1/8, shift=-2 folded in; shift cancels in softmax)
     and writes fp8 into e[b] [128(k), KT, 4(head), S].
  4. PV: per (head, qc) a 4-matmul fp8 DR chain with the [V|ones] stationary
     -> PSUM [128, 512] = attention numerator (rows 0-63) + denominator
     replicated (rows 64-127). No separate denominator matmuls.
  5. Normalize: ACT tensor-copy den rows -> SBUF partitions 0-63 (DVE cannot
     cross partitions except in tensor_copy; ACT copy keeps DVE free), DVE
     reciprocal_approx_fast, DVE multiply -> fp8 at[b] [64, 4(head), S].
  6. proj 1x1 conv fp8 DoubleRow over head-pair planes + residual + bias
     fused into the PSUM->SBUF evacuation.

HW notes baked in: DR needs fp8 and dst partition 0; 32-row stationaries may
sit at tile positions 0/32/64/96 and stream concurrently; DVE/ACT ops other
than tensor_copy reject partition-offset-mismatched operands; DVE reads at
most one PSUM operand; ACT can read a [128, 1024] AP spanning two PSUM banks.
"""

import sys

sys.path.insert(0, "/opt/trn_rl_repo")

from contextlib import ExitStack

import numpy as np
import ml_dtypes

import concourse.bass as bass
import concourse.tile as tile
from concourse import bacc, mybir
from concourse.bass_utils import run_bass_kernel_spmd

F32 = mybir.dt.float32
BF16 = mybir.dt.bfloat16
FP8 = mybir.dt.float8e4
I32 = mybir.dt.int32
I16 = mybir.dt.int16
U8 = mybir.dt.uint8
AF = mybir.ActivationFunctionType
OP = mybir.AluOpType
DR = mybir.MatmulPerfMode.DoubleRow

N_CORES = 8
B_PER = 2          # batch elements per core
C = 256
S = 1024           # H*W
NH = 4
D = 64             # head dim
EPS = 1e-5
CT = C // 128      # channel planes (2)
KT = S // 128      # key tiles (8)
QC = S // 512      # q chunks of 512 (2)
SHIFT = -2.0       # exp(s/8 + SHIFT); cancels in softmax, keeps fp8 in range
MAGIC = 0x5F3759DF
NCOLS = 4 + 2 + 2 + 2 + 2 * C + 128   # packed const columns
NWARM = 12
# Schraudolph exp-as-fp8e4m3-bits: u8_bits(exp(s/8 + SHIFT)) ~= s*SCH_A + SCH_B
# (negatives saturate to 0 on the f32->u8 convert = fp8 +0.0)
SCH_A = 0.125 * 8 * 1.4426950408889634
SCH_B = 8.0 * (7.0 + SHIFT * 1.4426950408889634)
OFFLOAD_T = (1, 3, 5, 7)   # t values whose hp1 exp runs on DVE instead of ACT


def build_nc():
    nc = bacc.Bacc("TRN2", target_bir_lowering=False, debug=False,
                   num_devices=N_CORES)

    x_d = nc.dram_tensor("x", [B_PER, C, S], F32, kind="ExternalInput").ap()
    wqkvT_d = nc.dram_tensor("wqkvT", [128, 2 * 3 * C], FP8, kind="ExternalInput").ap()
    wprojT_d = nc.dram_tensor("wprojT", [64, 4 * C], FP8, kind="ExternalInput").ap()
    cst_d = nc.dram_tensor("cst", [128, NCOLS], F32, kind="ExternalInput").ap()
    ones_d = nc.dram_tensor("ones8", [128, 2048], FP8, kind="ExternalInput").ap()
    out_d = nc.dram_tensor("out", [B_PER, C, S], F32, kind="ExternalOutput").ap()

    with tile.TileContext(nc) as tc, ExitStack() as ctx:
        cpool = ctx.enter_context(tc.tile_pool(name="consts", bufs=1))
        xpool = ctx.enter_context(tc.tile_pool(name="x", bufs=1))
        hnpool = ctx.enter_context(tc.tile_pool(name="hn", bufs=1))
        qkpool = ctx.enter_context(tc.tile_pool(name="qk", bufs=1))
        vtpool = ctx.enter_context(tc.tile_pool(name="vt", bufs=1))
        expool = ctx.enter_context(tc.tile_pool(name="expS", bufs=1))
        atpool = ctx.enter_context(tc.tile_pool(name="attn", bufs=1))
        dnpool = ctx.enter_context(tc.tile_pool(name="den", bufs=3))
        opool = ctx.enter_context(tc.tile_pool(name="osb", bufs=2))
        scpool = ctx.enter_context(tc.tile_pool(name="scratch", bufs=1))
        vecpool = ctx.enter_context(tc.tile_pool(name="vec", bufs=2))

        ps_sc = ctx.enter_context(tc.tile_pool(name="ps_sc", bufs=2,
                                               space="PSUM"))
        ps_pv = ctx.enter_context(tc.tile_pool(name="ps_pv", bufs=2,
                                               space="PSUM"))
        ps_qkv = ctx.enter_context(tc.tile_pool(name="ps_qkv", bufs=2,
                                                space="PSUM"))

        # ---- inputs: x first (GN needs it), weights/consts on spare queues
        xt = {}      # (b, ct) -> x tile [128, 1024] f32
        for b in range(B_PER):
            for ct in range(CT):
                xt[(b, ct)] = xpool.tile([128, 1024], F32, name=f"x{b}{ct}",
                                         tag=f"x{b}{ct}")
        # batch-0 x split into halves across both queues: GN's first
        # bn_stats only needs a [128, 512] half, so stats start ~2.5us in
        nc.sync.dma_start(xt[(0, 0)][:, 0:512], x_d[0, 0:128, 0:512])
        nc.gpsimd.dma_start(xt[(0, 1)][:, 0:512], x_d[0, 128:256, 0:512])
        nc.sync.dma_start(xt[(0, 0)][:, 512:1024], x_d[0, 0:128, 512:1024])
        nc.gpsimd.dma_start(xt[(0, 1)][:, 512:1024], x_d[0, 128:256, 512:1024])

        wq = cpool.tile([128, 2, 3 * C], FP8, name="wq", tag="wq")
        cst = cpool.tile([128, NCOLS], F32, name="cst", tag="cst")
        wp = cpool.tile([64, 4, C], FP8, name="wp", tag="wp")
        nc.scalar.dma_start(cst[:], cst_d[:])
        nc.scalar.dma_start(wq[:], wqkvT_d[:])
        nc.scalar.dma_start(wp[:], wprojT_d[:])
        nc.sync.dma_start(xt[(1, 0)][:], x_d[1, 0:128, :])
        nc.gpsimd.dma_start(xt[(1, 1)][:], x_d[1, 128:256, :])

        qkb = cst[:, 0:4]            # Q z0, Q z1, K z0, K z1 (permuted order)
        pb = cst[:, 4:6]
        nw = cst[:, 6:8]
        nb = cst[:, 8:10]
        bv2 = cst[:, 10:10 + 2 * C]
        G = cst[:, 10 + 2 * C:10 + 2 * C + 128]

        magic = cpool.tile([128, 2], I32, name="magic", tag="magic")
        nc.gpsimd.memset(magic[:], MAGIC)
        shiftc = cpool.tile([128, 1], F32, name="shiftc", tag="shiftc")
        nc.gpsimd.memset(shiftc[:], SHIFT)

        # per-batch state
        hnt = {}     # b -> hn tile [128, CT, 1024] fp8
        qQ = {}      # b -> Q' [128, 2(z), 1024] fp8
        qK = {}      # b -> K' [128, 2(z), 1024] fp8
        vtt = {}     # b -> v^T [128, KT, 4(head), 128] fp8; cols 64:128 ones
        ext = {}     # b -> exp [128, KT, 4(head), 1024] fp8
        att = {}     # b -> at [64, 4(head), 1024] fp8

        # vt tiles + their ones blocks (DMA'd early, off engines)
        for b in range(B_PER):
            vtt[b] = vtpool.tile([128, KT, 4, 128], FP8, name=f"vt{b}",
                                 tag=f"vt{b}")
            nc.gpsimd.dma_start(vtt[b][:, :, :, 64:128], ones_d[:])

        # dead warmup matmuls ramp the PE clock while GN runs. The warm tile
        # is intentionally NOT initialized (results are discarded) so the
        # warmups can issue immediately instead of waiting on a memset
        # behind the DMA queue.
        warm = scpool.tile([128, 2, 256], FP8, name="warm", tag="warm")
        nc.vector.memset(warm[:, 0, 0:1], 0.125)   # writer so deps resolve
        for w in range(NWARM):
            wps = ps_qkv.tile([128, 512], F32, name=f"wps{w}", tag="qkv")
            nc.tensor.matmul(out=wps[:, 0:256], lhsT=warm[:, :, 0:128],
                             rhs=warm[:],
                             start=True, stop=True, perf_mode=DR)

        def emit_gn(b):
            """GroupNorm stats + apply for batch b."""
            stats = vecpool.tile([128, 4], F32, name=f"st{b}", tag="stats")
            nvar = vecpool.tile([128, 2], F32, name=f"nv{b}", tag="nvar")
            veps = vecpool.tile([128, 2], F32, name=f"ve{b}", tag="veps")
            yis = vecpool.tile([128, 2], I32, name=f"yi{b}", tag="yis")
            rstd = vecpool.tile([128, 2], F32, name=f"rs{b}", tag="rstd")
            hneg = vecpool.tile([128, 2], F32, name=f"hg{b}", tag="hneg")
            tsq = vecpool.tile([128, 2], F32, name=f"tq{b}", tag="tsq")
            usq = vecpool.tile([128, 2], F32, name=f"uq{b}", tag="usq")
            Av = vecpool.tile([128, 2], F32, name=f"A{b}", tag="Av")
            nBv = vecpool.tile([128, 2], F32, name=f"nB{b}", tag="nBv")
            gsb = vecpool.tile([128, 4], F32, name=f"gs{b}", tag="gsb")
            bst = vecpool.tile([128, CT, 12], F32, name=f"bs{b}", tag="bst")
            agg = vecpool.tile([128, 4], F32, name=f"ag{b}", tag="agg")
            for ct in range(CT):
                for h in range(2):
                    nc.vector.bn_stats(
                        out=bst[:, ct, 6 * h:6 * h + 6],
                        in_=xt[(b, ct)][:, 512 * h:512 * h + 512])
                nc.vector.bn_aggr(out=agg[:, 2 * ct:2 * ct + 2],
                                  in_=bst[:, ct, :])
                nc.vector.tensor_copy(stats[:, ct:ct + 1],
                                      agg[:, 2 * ct:2 * ct + 1])
                nc.vector.scalar_tensor_tensor(
                    out=stats[:, 2 + ct:3 + ct], in0=agg[:, 2 * ct:2 * ct + 1],
                    scalar=agg[:, 2 * ct:2 * ct + 1],
                    in1=agg[:, 2 * ct + 1:2 * ct + 2],
                    op0=OP.mult, op1=OP.add)
            gps = ps_qkv.tile([128, 4], F32, name=f"g{b}", tag="qkv")
            nc.tensor.matmul(out=gps[:], lhsT=G, rhs=stats[:],
                             start=True, stop=True)
            nc.vector.tensor_copy(gsb[:], gps[:])
            means = gsb[:, 0:2]
            e2s = gsb[:, 2:4]
            nc.vector.tensor_tensor(out=nvar[:], in0=means, in1=means,
                                    op=OP.mult)
            nc.vector.tensor_tensor(out=nvar[:], in0=nvar[:], in1=e2s,
                                    op=OP.subtract)
            nc.vector.tensor_scalar(
                out=veps[:], in0=nvar[:], scalar1=-1.0, scalar2=EPS,
                op0=OP.mult, op1=OP.add)
            nc.vector.tensor_scalar(
                out=yis[:], in0=veps[:].bitcast(I32), scalar1=1, scalar2=None,
                op0=OP.arith_shift_right)
            nc.vector.tensor_tensor(
                out=yis[:], in0=magic[:], in1=yis[:], op=OP.subtract)
            y = yis[:].bitcast(F32)
            nc.vector.tensor_scalar(
                out=hneg[:], in0=veps[:], scalar1=-0.5, scalar2=None,
                op0=OP.mult)
            nc.vector.tensor_tensor(out=tsq[:], in0=y, in1=y, op=OP.mult)
            nc.vector.tensor_tensor(out=usq[:], in0=tsq[:], in1=hneg[:],
                                    op=OP.mult)
            nc.vector.scalar_tensor_tensor(
                out=rstd[:], in0=usq[:], scalar=1.5, in1=y,
                op0=OP.add, op1=OP.mult)
            nc.vector.tensor_mul(Av[:], rstd[:], nw)
            nc.vector.tensor_tensor(out=nBv[:], in0=means, in1=Av[:],
                                    op=OP.mult)
            nc.vector.tensor_tensor(out=nBv[:], in0=nBv[:], in1=nb,
                                    op=OP.subtract)
            hn = hnpool.tile([128, CT, 1024], FP8, name=f"hn{b}", tag=f"hn{b}")
            hnt[b] = hn
            # qc0 halves of both planes first: the first QKV matmuls
            # (q-chunk 0) unblock after two half-applies
            for half in range(2):
                hs = slice(512 * half, 512 * (half + 1))
                for ct in range(CT):
                    nc.vector.tensor_scalar(
                        out=hn[:, ct, hs], in0=xt[(b, ct)][:, hs],
                        scalar1=Av[:, ct:ct + 1],
                        scalar2=nBv[:, ct:ct + 1], op0=OP.mult,
                        op1=OP.subtract)

        def qk_chunks(b, qc, act_evac=False):
            """Emit the 4 QKV matmuls (K z0, K z1, Q z0, Q z1) for q-chunk
            qc. K first: scores t<4 need only the qc0 K columns. With
            act_evac, alternate evacuations ACT/DVE (startup: ACT is idle
            and halves the serial evac leg before the first scores)."""
            if b not in qQ:
                qQ[b] = qkpool.tile([128, 2, 1024], FP8, name=f"qQ{b}",
                                    tag=f"qQ{b}")
                qK[b] = qkpool.tile([128, 2, 1024], FP8, name=f"qK{b}",
                                    tag=f"qK{b}")
            qs = slice(512 * qc, 512 * (qc + 1))
            for n, (dst, j) in enumerate(((qK[b], 2), (qK[b], 3),
                                          (qQ[b], 0), (qQ[b], 1))):
                z = j % 2
                ps = ps_qkv.tile([128, 512], F32, name=f"qp{b}{j}{qc}",
                                 tag="qkv")
                nc.tensor.matmul(
                    out=ps[:],
                    lhsT=wq[:, :, 128 * j:128 * (j + 1)],
                    rhs=hnt[b][:, :, qs],
                    start=True, stop=True, perf_mode=DR)
                if act_evac and n % 2 == 0:
                    nc.scalar.activation(dst[:, z, qs], ps[:], AF.Identity,
                                         bias=qkb[:, j:j + 1], scale=1.0)
                else:
                    nc.vector.tensor_scalar(
                        out=dst[:, z, qs], in0=ps[:],
                        scalar1=qkb[:, j:j + 1], scalar2=None, op0=OP.add)
                yield

        def v_chunks(b):
            """V^T into vt[b] head blocks (cols 0:64 of each 128-block)."""
            vt = vtt[b]
            for tp in range(0, KT, 2):
                ps = ps_qkv.tile([128, 2, 4, 64], F32, name=f"vp{b}{tp}",
                                 tag="qkv")
                for i in range(2):
                    nc.tensor.matmul(
                        out=ps[:, i],
                        lhsT=hnt[b][:, :, 128 * (tp + i):128 * (tp + i + 1)],
                        rhs=wq[:, :, 512:768],
                        start=True, stop=True, perf_mode=DR)
                nc.vector.scalar_tensor_tensor(
                    out=vt[:, tp:tp + 2, :, 0:64], in0=ps[:], scalar=1.0,
                    in1=bv2, op0=OP.bypass, op1=OP.add)
                yield

        def scores_chunks(b, qc, hp_major=False):
            """Scores+exp for q-chunk qc. Default: per t a quad of four
            concurrent 32-row DR matmuls (both head pairs) into two wide
            PSUM units, then the two exps (ACT; hp1 units with t in
            OFFLOAD_T run on DVE as Schraudolph fp8-bit exps, in parallel
            with ACT on the hp0 unit). hp_major orders head-pair-outer
            (pairs only) so the tail's PV chains can start mid-stream."""
            if b not in ext:
                ext[b] = expool.tile([128, KT, 4, 1024], FP8, name=f"ex{b}",
                                     tag=f"ex{b}")
            e = ext[b]
            qs = slice(512 * qc, 512 * (qc + 1))

            def mm(t, hp, ch):
                for i in range(2):
                    h = 2 * hp + i
                    rows = slice(32 * h, 32 * h + 32)
                    nc.tensor.matmul(
                        out=ch[:, 512 * i:512 * (i + 1)],
                        lhsT=qK[b][rows, :, 128 * t:128 * (t + 1)],
                        rhs=qQ[b][rows, :, qs],
                        start=True, stop=True, perf_mode=DR,
                        tile_position=(32 * h, 0))

            def ex(t, hp, ch):
                if hp == 1 and t in OFFLOAD_T:
                    nc.vector.tensor_scalar(
                        out=e[:, t, 2:4, qs].bitcast(U8), in0=ch[:],
                        scalar1=SCH_A, scalar2=SCH_B,
                        op0=OP.mult, op1=OP.add)
                else:
                    nc.scalar.activation(e[:, t, 2 * hp:2 * hp + 2, qs],
                                         ch[:], AF.Exp,
                                         bias=shiftc[:, 0:1], scale=0.125)

            if hp_major is not False:
                hps = (range(2) if isinstance(hp_major, bool)
                       else [hp_major])
                for hp in hps:
                    for t in range(KT):
                        ch = ps_sc.tile([128, 1024], F32,
                                        name=f"s{b}{qc}{t}{hp}", tag="sc")
                        mm(t, hp, ch)
                        ex(t, hp, ch)
                        yield
            else:
                for t in range(KT):
                    chs = []
                    for hp in range(2):
                        ch = ps_sc.tile([128, 1024], F32,
                                        name=f"s{b}{qc}{t}{hp}", tag="sc")
                        chs.append(ch)
                        mm(t, hp, ch)
                    for hp in range(2):
                        ex(t, hp, chs[hp])
                    yield
                    yield

        def pv_chunks(b, qc, heads=(0, 1, 2, 3)):
            """PV chains + normalize for q-chunk qc, in head pairs sharing
            one wide sc-pool tile (chain h in cols 0:512, h+1 in 512:1024).

            Each head: 4 fp8 DR matmuls with the [V|ones] stationary
            accumulate numerator (PSUM rows 0-63) and denominator
            (replicated over rows 64-127); then the denominator is copied
            to SBUF partitions 0-63 (ACT/DVE alternating; only tensor_copy
            may cross partitions), DVE reciprocal + multiply -> at[b].
            """
            if b not in att:
                att[b] = atpool.tile([64, 4, 1024], FP8, name=f"at{b}",
                                     tag=f"at{b}")
            e, vt, at = ext[b], vtt[b], att[b]
            qs = slice(512 * qc, 512 * (qc + 1))
            for h in heads:
                u = ps_pv.tile([128, 512], F32, name=f"u{b}{qc}{h}",
                               tag="pv")
                for tp in range(0, KT, 2):
                    nc.tensor.matmul(
                        out=u[:], lhsT=vt[:, tp:tp + 2, h, :],
                        rhs=e[:, tp:tp + 2, h, qs],
                        start=(tp == 0), stop=(tp == KT - 2),
                        perf_mode=DR,
                        tile_position=(0, 0), skip_group_check=True)
                    if tp == 2:
                        yield
                dn = dnpool.tile([64, 512], F32, name=f"dn{b}{qc}{h}",
                                 tag="dn")
                rc = dnpool.tile([64, 512], F32, name=f"rc{b}{qc}{h}",
                                 tag="rc")
                if h % 2 == 0:
                    nc.scalar.copy(dn[:], u[64:128, :])
                else:
                    nc.vector.tensor_copy(dn[:], u[64:128, :])
                yield
                nc.vector.reciprocal_approx_fast(rc[:], dn[:])
                nc.vector.tensor_mul(at[:, h, qs], u[0:64, :], rc[:])
                yield

        def proj_chunks(b, qc):
            """proj (fp8 DR over head-pair planes) + residual + bias."""
            at = att[b]
            qs = slice(512 * qc, 512 * (qc + 1))
            for m in range(CT):
                ps = ps_qkv.tile([128, 512], F32, name=f"pj{b}{qc}{m}",
                                 tag="qkv")
                nc.tensor.matmul(
                    out=ps[:], lhsT=wp[:, 0:2, 128 * m:128 * (m + 1)],
                    rhs=at[:, 0:2, qs],
                    start=True, stop=False, perf_mode=DR)
                nc.tensor.matmul(
                    out=ps[:], lhsT=wp[:, 2:4, 128 * m:128 * (m + 1)],
                    rhs=at[:, 2:4, qs],
                    start=False, stop=True, perf_mode=DR)
                osb = opool.tile([128, 512], F32, name=f"o{b}{qc}{m}",
                                 tag="osb")
                nc.vector.scalar_tensor_tensor(
                    out=osb[:], in0=ps[:], scalar=pb[:, m:m + 1],
                    in1=xt[(b, m)][:, qs], op0=OP.add, op1=OP.add)
                nc.sync.dma_start(out_d[b, 128 * m:128 * (m + 1), qs],
                                  osb[:])
                yield

        def chain(*gens):
            for g in gens:
                yield from g

        def interleave(lead, filler):
            lead, filler = iter(lead), iter(filler)
            while True:
                stop = 0
                for g in (lead, filler):
                    try:
                        next(g)
                    except StopIteration:
                        stop += 1
                if stop == 2:
                    return

        def drain(g):
            for _ in g:
                pass

        # ---- software-pipelined emission ----
        emit_gn(0)
        drain(qk_chunks(0, 0))
        emit_gn(1)
        interleave(scores_chunks(0, 0),
                   chain(qk_chunks(0, 1), v_chunks(0)))
        interleave(scores_chunks(0, 1),
                   chain(qk_chunks(1, 0), pv_chunks(0, 0),
                         qk_chunks(1, 1)))
        interleave(scores_chunks(1, 0),
                   chain(v_chunks(1), pv_chunks(0, 1), proj_chunks(0, 0),
                         proj_chunks(0, 1)))
        interleave(scores_chunks(1, 1, hp_major=True),
                   chain(pv_chunks(1, 0), proj_chunks(1, 0),
                         pv_chunks(1, 1, heads=(0, 1))))
        drain(chain(pv_chunks(1, 1, heads=(2, 3)), proj_chunks(1, 1)))

    nc.compile()
    return nc


_NC = None


def _get_nc():
    global _NC
    if _NC is None:
        _NC = build_nc()
    return _NC


def make_in_maps(x, norm_w, norm_b, qkv_w, qkv_b, proj_w, proj_b):
    x = np.asarray(x, dtype=np.float32)
    B = x.shape[0]
    assert B == N_CORES * B_PER

    # Q/K output-channel permutation: QK tile j in {Qz0, Qz1, Kz0, Kz1};
    # col 32h+p of tile (T, z) -> channel T_base + 64h + 32z + p
    perms = []
    for tbase in (0, C):          # Q then K
        for z in range(2):
            p = np.arange(32)
            cols = np.concatenate([tbase + 64 * h + 32 * z + p
                                   for h in range(NH)])
            perms.append(cols)
    # wq tile order: [Qz0, Qz1, Kz0, Kz1] at cols 0..511, V at 512..767
    wqf = np.asarray(qkv_w, np.float32)          # [3C, C]
    wnew = np.zeros((128, 2, 3 * C), np.float32)
    for j, cols in enumerate(perms):
        for ct in range(2):
            wnew[:, ct, 128 * j:128 * (j + 1)] = wqf[cols, 128 * ct:128 * ct + 128].T
    for ct in range(2):
        wnew[:, ct, 512:768] = wqf[2 * C:3 * C, 128 * ct:128 * ct + 128].T
    wqkvT = np.ascontiguousarray(
        wnew.reshape(128, 2 * 3 * C)).astype(ml_dtypes.float8_e4m3)

    # proj weights [64(d), 4(head), C] fp8
    wpf = np.asarray(proj_w, np.float32)         # [C, C] = [out, in]
    wprojT = np.ascontiguousarray(
        np.stack([wpf[:, 64 * h:64 * h + 64].T for h in range(NH)], axis=1)
        .reshape(64, 4 * C)).astype(ml_dtypes.float8_e4m3)

    qbf = np.asarray(qkv_b, np.float32)
    cst = np.zeros((128, NCOLS), np.float32)
    for j, cols in enumerate(perms):
        cst[:, j] = qbf[cols]
    cst[:, 4:6] = np.asarray(proj_b, np.float32).reshape(2, 128).T
    cst[:, 6:8] = np.asarray(norm_w, np.float32).reshape(2, 128).T
    cst[:, 8:10] = np.asarray(norm_b, np.float32).reshape(2, 128).T
    cst[:, 10:10 + 2 * C] = np.broadcast_to(
        np.tile(qbf[512:768], 2), (128, 2 * C))
    G = np.zeros((128, 128), np.float32)
    for g in range(4):
        G[32 * g:32 * (g + 1), 32 * g:32 * (g + 1)] = 1.0 / 32.0
    cst[:, 10 + 2 * C:10 + 2 * C + 128] = G

    ones8 = np.ones((128, 2048), dtype=ml_dtypes.float8_e4m3)

    xs = x.reshape(N_CORES, B_PER, C, S)
    common = dict(wqkvT=wqkvT, wprojT=wprojT, cst=cst, ones8=ones8)
    return [dict(x=np.ascontiguousarray(xs[i]), **common)
            for i in range(N_CORES)]


def kernel(x, norm_w, norm_b, qkv_w, qkv_b, proj_w, proj_b):
    in_maps = make_in_maps(x, norm_w, norm_b, qkv_w, qkv_b, proj_w, proj_b)
    nc = _get_nc()
    res = run_bass_kernel_spmd(nc, in_maps, core_ids=list(range(N_CORES)))
    out = np.stack([res.results[i]["out"] for i in range(N_CORES)], axis=0)
    return out.reshape(x.shape[0], C, 32, 32).astype(np.float32)
